# revision 42
# baseline (speedup 1.0000x reference)
"""Multi-head causal attention (b=1, s=4096, d=1024, 16 heads) on 8 NeuronCores.

Sharding: tensor-parallel over heads — 2 heads per core. Each core computes
Q/K/V projections for its heads, causal attention, and its row-slice of the
output projection (partial sum). Host sums the 8 partial outputs.

Device layout notes:
 - x is pre-transposed + bf16-cast on host: xT [1024, 4096] so every matmul
   sees the contraction dim on partitions with no on-chip transposes.
 - Scores are computed transposed (S^T [k, q]) so the probs tile is already
   the PV matmul's moving operand; softmax denominator comes from a ones
   column appended to V (row 64 of the PV accumulator).
 - exp has no max-subtraction: scores ~ N(0,1) by construction, fp32 PSUM
   holds exp(s/8) easily.
 - Causal masking is multiplicative on exp(S^T) using a precomputed
   [128, 1024] sliding mask (only diagonal blocks need it).
"""

import numpy as np
import ml_dtypes

import concourse.bass as bass
import concourse.mybir as mybir
import concourse.tile as tile
from concourse import bacc
from concourse.bass_utils import run_bass_kernel_spmd

BF16 = ml_dtypes.bfloat16
S = 4096          # sequence length
D = 1024          # model dim
NCORES = 8
HL = 2            # heads per core
HD = 64           # head dim
DK = D // 128     # 8 contraction tiles for projections
NQC = S // 512    # 8 query chunks of 512
NKT = S // 128    # 32 key tiles of 128
FP32 = mybir.dt.float32
BF = mybir.dt.bfloat16
EXP = mybir.ActivationFunctionType.Exp


def _build_program(repeat=1):
    nc = bacc.Bacc("TRN2", target_bir_lowering=False, debug=False, num_devices=NCORES)

    xT = nc.dram_tensor("xT", [D, S], BF, kind="ExternalInput").ap()
    wq = nc.dram_tensor("wq", [D, 128], BF, kind="ExternalInput").ap()
    wk = nc.dram_tensor("wk", [D, 128], BF, kind="ExternalInput").ap()
    wv = nc.dram_tensor("wv", [D, 128], BF, kind="ExternalInput").ap()
    wo = nc.dram_tensor("wo", [128, D], BF, kind="ExternalInput").ap()
    mask = nc.dram_tensor("mask", [128, 1024], BF, kind="ExternalInput").ap()
    y = nc.dram_tensor("y", [S, D], FP32, kind="ExternalOutput").ap()

    with tile.TileContext(nc) as tc:
        with (
            tc.tile_pool(name="persist", bufs=1) as pp,
            tc.tile_pool(name="stp", bufs=2, space="PSUM") as stp,
            tc.tile_pool(name="sdp", bufs=2, space="PSUM") as sdp,
            tc.tile_pool(name="otp", bufs=2, space="PSUM") as otp,
            tc.tile_pool(name="epool", bufs=10) as ep,
            tc.tile_pool(name="small", bufs=4) as sp,
            tc.tile_pool(name="ystage", bufs=6) as ysp,
        ):
            # ---- persistent SBUF tiles ----
            # chunk c holds all 8 D-row-blocks of xT for seq cols
            # [512c, 512c+512): block i at free cols [512i, 512i+512)
            xt = [pp.tile([128, DK * 512], BF, tag=f"xt{c}", name=f"xt{c}")
                  for c in range(NQC)]
            wq_sb = pp.tile([128, D], BF, tag="wq")
            wk_sb = pp.tile([128, D], BF, tag="wk")
            wv_sb = pp.tile([128, D], BF, tag="wv")
            wo_sb = pp.tile([128, D], BF, tag="wo")
            mask_sb = pp.tile([128, 1024], BF, tag="mask")
            qT = [pp.tile([128, 512], BF, tag=f"qT{c}", name=f"qT{c}") for c in range(NQC)]
            kT = [pp.tile([128, 512], BF, tag=f"kT{c}", name=f"kT{c}") for c in range(NQC)]
            # V augmented with a ones column, grouped 4 k-tiles per tile:
            # group g holds k-tiles 4g..4g+3; slice ((kt%4)*HL + h)*65
            vaug = [pp.tile([128, 4 * HL * 65], BF, tag=f"va{g}", name=f"va{g}") for g in range(NKT // 4)]
            # normalized attention output O^T, chunked by query chunk
            ot_sb = [pp.tile([128, 512], BF, tag=f"ot{c}", name=f"ot{c}") for c in range(NQC)]

            # ---- input DMAs: weights first (small, needed immediately) ----
            for w_sb, w_dram in ((wk_sb, wk), (wq_sb, wq), (wv_sb, wv)):
                nc.sync.dma_start(
                    out=w_sb[:].rearrange("p (i j) -> p i j", i=DK),
                    in_=w_dram.rearrange("(i p) j -> p i j", p=128),
                )
            nc.sync.dma_start(out=mask_sb[:], in_=mask[:])
            nc.sync.dma_start(out=wo_sb[:], in_=wo[:])

            xTr = xT.rearrange("(i p) s -> p i s", p=128)

            def load_x_chunk(c, split=False):
                if split:
                    # per-D-tile DMAs so the first projection's matmul i can
                    # start as soon as block i lands
                    for i in range(DK):
                        nc.sync.dma_start(
                            out=xt[c][:, 512 * i:512 * (i + 1)],
                            in_=xT[128 * i:128 * (i + 1), 512 * c:512 * (c + 1)],
                        )
                else:
                    nc.sync.dma_start(
                        out=xt[c][:].rearrange("p (i s) -> p i s", i=DK),
                        in_=xTr[:, :, 512 * c:512 * (c + 1)],
                    )

            load_x_chunk(0, split=True)
            load_x_chunk(1)
            load_x_chunk(2)

            # ones columns of vaug
            for g in range(NKT // 4):
                for j in range(4 * HL):
                    nc.gpsimd.memset(vaug[g][:, j * 65 + 64:j * 65 + 65], 1.0)

            # ---- per-chunk projections (emitted interleaved with attention) ----
            _qk_accs = {}

            def qk_half(qc, which, half):
                w_sb, dst = (wk_sb, kT[qc]) if which == "k" else (wq_sb, qT[qc])
                if half == 0:
                    acc = sdp.tile([128, 512], FP32, tag="sd", name="mmt")
                    _qk_accs[(qc, which)] = acc
                else:
                    acc = _qk_accs.pop((qc, which))
                for i in range(4 * half, 4 * half + 4):
                    nc.tensor.matmul(
                        acc[:],
                        w_sb[:, 128 * i:128 * (i + 1)],
                        xt[qc][:, 512 * i:512 * (i + 1)],
                        start=(i == 0),
                        stop=(i == DK - 1),
                    )
                if half == 1:
                    # round 0: ACT is idle until the first exp; keep DVE clear
                    nc.vector.tensor_copy(dst[:], acc[:])

            def v_group(kt):
                g = kt // 4
                acc = sdp.tile([128, 128], FP32, tag="sd", name="mmv")
                for i in range(DK):
                    nc.tensor.matmul(
                        acc[:],
                        xt[g][:, 512 * i + 128 * (kt % 4):512 * i + 128 * (kt % 4) + 128],
                        wv_sb[:, 128 * i:128 * (i + 1)],
                        start=(i == 0),
                        stop=(i == DK - 1),
                    )
                j = kt % 4
                for h in range(HL):
                    base = (j * HL + h) * 65
                    nc.vector.tensor_copy(
                        vaug[g][:, base:base + 64], acc[:, 64 * h:64 * h + 64]
                    )

            def proj_units(qc):
                units = [lambda w=w, hf=hf: qk_half(qc, w, hf)
                         for w in ("k", "q") for hf in (0, 1)]
                units += [lambda kt=kt: v_group(kt) for kt in range(4 * qc, 4 * qc + 4)]
                return units

            # ---- causal attention: both heads of one query chunk, with the
            # two heads' S->exp->PV pipelines interleaved stage by stage ----
            def attention(qc, fillers):
                ot_accs = {hh: otp.tile([65, 512], FP32, tag="ot",
                                        name=f"otacc{hh}") for hh in range(HL)}
                st_tiles = {}
                e_tiles = {}

                def kslice(h, kt):
                    return kT[kt // 4][
                        64 * h:64 * h + 64, 128 * (kt % 4):128 * (kt % 4) + 128
                    ]

                def vslice(h, kt):
                    base = ((kt % 4) * HL + h) * 65
                    return vaug[kt // 4][:, base:base + 65]

                # --- full (non-diagonal) k-tiles, processed in pairs:
                # two k-tiles side by side in one 2-bank PSUM tile so a
                # single exp instruction covers both
                def s_pair(h, p):
                    st = stp.tile([128, 1024], FP32, tag="st", name="stt")
                    for u in range(2):
                        nc.tensor.matmul(
                            st[:, 512 * u:512 * (u + 1)],
                            kslice(h, 2 * p + u),
                            qT[qc][64 * h:64 * h + 64, :],
                            start=True,
                            stop=True,
                        )
                    st_tiles[(h, p)] = st

                def exp_pair(h, p):
                    e = ep.tile([128, 1024], BF, tag="e", name="etile")
                    nc.scalar.activation(e[:], st_tiles.pop((h, p))[:], EXP,
                                         scale=0.125)
                    e_tiles[(h, p)] = e

                def pv_pair(h, p):
                    e = e_tiles.pop((h, p))
                    for u in range(2):
                        kt = 2 * p + u
                        nc.tensor.matmul(
                            ot_accs[h][:],
                            vslice(h, kt),
                            e[:, 512 * u:512 * (u + 1)],
                            start=False,
                            stop=(kt == 4 * qc - 1),
                            skip_group_check=True,
                        )

                # --- diagonal k-tiles, q-trimmed, two per ST tile so one
                # exp instruction covers both (each matmul stays in one bank)
                def dwidths(dp):
                    k0 = 4 * qc + 2 * dp
                    return k0, 512 - 128 * (2 * dp), 512 - 128 * (2 * dp + 1)

                def s_dpair(h, dp):
                    k0, w0, w1 = dwidths(dp)
                    st = stp.tile([128, w0 + w1], FP32, tag="st", name="stdp")
                    nc.tensor.matmul(
                        st[:, 0:w0],
                        kslice(h, k0),
                        qT[qc][64 * h:64 * h + 64, 512 - w0:512],
                        start=True,
                        stop=True,
                    )
                    nc.tensor.matmul(
                        st[:, w0:w0 + w1],
                        kslice(h, k0 + 1),
                        qT[qc][64 * h:64 * h + 64, 512 - w1:512],
                        start=True,
                        stop=True,
                    )
                    st_tiles[(h, "d", dp)] = st

                def exp_dpair(h, dp):
                    k0, w0, w1 = dwidths(dp)
                    e = ep.tile([128, w0 + w1], BF, tag="e", name="etiled")
                    nc.scalar.activation(e[:], st_tiles.pop((h, "d", dp))[:],
                                         EXP, scale=0.125)
                    nc.vector.tensor_mul(e[:, 0:w0], e[:, 0:w0],
                                         mask_sb[:, 512:512 + w0])
                    nc.vector.tensor_mul(e[:, w0:w0 + w1], e[:, w0:w0 + w1],
                                         mask_sb[:, 512:512 + w1])
                    e_tiles[(h, "d", dp)] = e

                def pv_dpair(h, dp):
                    k0, w0, w1 = dwidths(dp)
                    e = e_tiles.pop((h, "d", dp))
                    nc.tensor.matmul(
                        ot_accs[h][:, 512 - w0:512],
                        vslice(h, k0),
                        e[:, 0:w0],
                        start=(dp == 0),
                        stop=False,
                        skip_group_check=True,
                    )
                    nc.tensor.matmul(
                        ot_accs[h][:, 512 - w1:512],
                        vslice(h, k0 + 1),
                        e[:, w0:w0 + w1],
                        start=False,
                        stop=(qc == 0 and dp == 1),
                        skip_group_check=True,
                    )

                # software-pipelined emission: PE alternates S and PV so the
                # ACT exp latency is hidden; heads interleaved stage by stage
                per_head = [("d", dp) for dp in range(2)] + [
                    ("p", p) for p in range(2 * qc)]
                stages = []
                for st_ in per_head:
                    stages.append((0, *st_))
                    stages.append((1, *st_))
                emit_s = {"p": s_pair, "d": s_dpair}
                emit_e = {"p": exp_pair, "d": exp_dpair}
                emit_v = {"p": pv_pair, "d": pv_dpair}
                # normalize: rows 0..63 are O^T, row 64 is the denominator;
                # emitted per head as soon as its last PV has been issued so
                # the chain overlaps the other head's trailing stages
                def div(hh):
                    rd = sp.tile([1, 512], FP32, tag="rd", name="rdt")
                    nc.vector.reciprocal(rd[:], ot_accs[hh][64:65, :])
                    rdb = sp.tile([64, 512], FP32, tag="rdb", name="rdbt")
                    nc.gpsimd.partition_broadcast(rdb[:], rd[:])
                    nc.vector.tensor_mul(
                        ot_sb[qc][64 * hh:64 * hh + 64, :],
                        ot_accs[hh][0:64, :], rdb[:]
                    )

                last_stage = {hh: max(i for i, s_ in enumerate(stages)
                                      if s_[0] == hh) for hh in range(HL)}
                nstages = len(stages)
                for i, (hh, kind, idx) in enumerate(stages):
                    emit_s[kind](hh, idx)
                    if i > 0:
                        ph, pk, pi = stages[i - 1]
                        emit_v[pk](ph, pi)
                        if i - 1 == last_stage[ph]:
                            div(ph)
                    emit_e[kind](hh, idx)
                    # spread remaining fillers evenly over remaining stages
                    rem = nstages - i
                    if fillers and len(fillers) >= rem:
                        for _ in range(-(-len(fillers) // rem)):
                            if fillers:
                                fillers.pop(0)()
                    elif fillers and (i * len(fillers)) // nstages != ((i + 1) * len(fillers)) // nstages:
                        fillers.pop(0)()
                lh, lk, li = stages[-1]
                emit_v[lk](lh, li)
                div(lh)

            def wo_unit(qc, t, n):
                qt = 4 * qc + t
                acc = sdp.tile([128, 512], FP32, tag="sd", name="yacc")
                nc.tensor.matmul(
                    acc[:],
                    ot_sb[qc][:, 128 * t:128 * (t + 1)],
                    wo_sb[:, 512 * n:512 * (n + 1)],
                    start=True,
                    stop=True,
                )
                ys = ysp.tile([128, 512], FP32, tag="ys", name="yst")
                # last chunk: ACT is idle at the tail while DVE runs the divs
                nc.vector.tensor_copy(ys[:], acc[:])
                nc.sync.dma_start(
                    out=y[128 * qt:128 * (qt + 1), 512 * n:512 * (n + 1)],
                    in_=ys[:],
                )

            def wo_units(qc):
                return [lambda t=t, n=n: wo_unit(qc, t, n)
                        for t in range(4) for n in range(2)]

          for _rep in range(repeat):
            for u in proj_units(0):
                u()
            for qc in range(NQC):
                if qc + 3 < NQC:
                    load_x_chunk(qc + 3)   # prefetch 3 chunks ahead
                fillers = []
                if qc + 1 < NQC:
                    fillers += proj_units(qc + 1)
                if qc >= 1:
                    fillers += wo_units(qc - 1)
                attention(qc, 0, fillers)
                attention(qc, 1, fillers)
                for u in fillers:   # drain leftovers
                    u()
                fillers.clear()
            for u in wo_units(NQC - 1):
                u()

    nc.compile()
    return nc


_program = None


def _get_program():
    global _program
    if _program is None:
        _program = _build_program()
    return _program


def _make_mask():
    t = np.arange(1024)[None, :]
    k = np.arange(128)[:, None]
    return (k <= t - 512).astype(BF16)


def kernel(x, Wq, Wk, Wv, Wo):
    x = np.asarray(x, dtype=np.float32)
    Wq, Wk, Wv, Wo = (np.asarray(w, dtype=np.float32) for w in (Wq, Wk, Wv, Wo))
    nc = _get_program()

    xT = np.ascontiguousarray(x[0].T).astype(BF16)
    mask = _make_mask()
    in_maps = []
    for c in range(NCORES):
        hs = slice(128 * c, 128 * (c + 1))
        in_maps.append({
            "xT": xT,
            "wq": np.ascontiguousarray(Wq[:, hs]).astype(BF16),
            "wk": np.ascontiguousarray(Wk[:, hs]).astype(BF16),
            "wv": np.ascontiguousarray(Wv[:, hs]).astype(BF16),
            "wo": np.ascontiguousarray(Wo[hs, :]).astype(BF16),
            "mask": mask,
        })

    res = run_bass_kernel_spmd(nc, in_maps, core_ids=list(range(NCORES)))
    out = np.zeros((S, D), np.float32)
    for c in range(NCORES):
        out += np.asarray(res.results[c]["y"], dtype=np.float32)
    return out.reshape(1, S, D)
